# revision 20
# baseline (speedup 1.0000x reference)
"""NT-Xent (SimCLR) loss on 8 Trainium2 NeuronCores.

Math (validated against the reference formulation in f64):
  z = concat(z_i, z_j)                      [N=4096, D=512]
  zn = z / max(||z||, eps);  sim = zn@zn.T / T   (T=0.5, sim in [-2, 2])
  logits row i = sim row i minus the self-diagonal entry, so
    lse_i  = log(sum_{j!=i} exp(sim_ij))          (sim bounded => no shift)
    loss   = mean_i (lse_i - sim[i, partner(i)])
    rank_i = #{j != i : sim_ij > sim[i, partner(i)]}  (partner ties at 0)
    avg_rank = mean_i rank_i

Sharding: core r owns 512 rows of sim. Host normalizes z, quantizes
zn*S8 to fp8e4m3 (PE fp8 matmul = 2x bf16 throughput; rank/loss error
~1e-4, validated), transposes to [D, N] and permutes columns per core
to [partner-block | own-block | rest] so the partner / self diagonals
sit at fixed compile-time offsets (one NEFF for all cores; row stats
are column-permutation invariant). The gram G = (S8 zn)(S8 zn)^T is
sim * S8^2/  (1/T); the device folds k = (1/T)/S8^2 into the ScalarE
activation scale, so exp(k*G) = exp(sim).

Per 1024-col PSUM pair (2 banks): 8 accumulating matmuls, one ScalarE
exp with fused row-sum (accum_out), one VectorE greater-than count with
fused row-sum, comparing in the exp domain against exp(k*pos) so the
compare reads f32 SBUF in the DVE 2x port mode instead of PSUM at 1x.
Diagonals (pos, self) are extracted exactly via identity-mask multiply
+ reduce from the first PSUM pair. Device emits per-row (S_full, d_G,
pos_G, count) in G units; the host applies the exact self-exclusion
corrections and the final log/mean in f64.
"""

import numpy as np
import ml_dtypes

import concourse.bacc as bacc
import concourse.mybir as mybir
import concourse.tile as tile
from concourse.bass_utils import run_bass_kernel_spmd
from concourse.masks import make_identity

B = 2048
D = 512
N = 2 * B
NCORES = 8
RPC = N // NCORES  # rows of sim per core = 512
KT = D // 128      # k tiles = 4
MT = RPC // 128    # m tiles per core = 4
PAIR = 1024        # columns per PSUM pair-tile (2 banks of f32)
NPAIR = N // PAIR  # pairs per m-tile = 4
S8 = 16.0          # fp8 pre-scale on zn
KSCALE = (1.0 / 0.5) / (S8 * S8)  # sim = KSCALE * G   (T = 0.5)

_f32 = mybir.dt.float32
_bf16 = mybir.dt.bfloat16
_fp8 = mybir.dt.float8e4

_NC_CACHE = {}


def _emit(tc):
    nc = tc.nc
    rhs_d = nc.dram_tensor("rhs", [KT, 128, N], _fp8, kind="ExternalInput")[:]
    out_d = nc.dram_tensor("out", [128, 4 * MT + 1], _f32, kind="ExternalOutput")[:]

    with (
        tc.tile_pool(name="singles", bufs=1) as singles,
        tc.tile_pool(name="psum", bufs=3, space="PSUM") as psum,
        tc.tile_pool(name="scratch", bufs=3) as scratch,
        tc.tile_pool(name="acc", bufs=4) as acc,
    ):
        ident = singles.tile([128, 128], _f32)
        make_identity(nc, ident)
        kscale = singles.tile([128, 1], _f32)
        nc.vector.memset(kscale, KSCALE)

        # Stage the full [D, N] fp8 operand in SBUF: 4 k-tiles of
        # [128, 4096] (4 KiB/partition each). Split DMAs column-wise so
        # the first matmuls can start before the tail columns land.
        rhs_sb = []
        for k in range(KT):
            t = singles.tile([128, N], _fp8, tag=f"rhs{k}")
            for q in range(4):
                nc.sync.dma_start(
                    out=t[:, q * 1024 : (q + 1) * 1024],
                    in_=rhs_d[k, :, q * 1024 : (q + 1) * 1024],
                )
            rhs_sb.append(t)

        # Spare column 16 is written once and ignored by the host.
        outs = singles.tile([128, 4 * MT + 1], _f32)
        nc.vector.tensor_scalar_mul(outs[:, 4 * MT : 4 * MT + 1], ident[:, 0:1], 0.0)

        for t in range(MT):
            pos = acc.tile([128, 1], _f32, tag="pos")
            dself = acc.tile([128, 1], _f32, tag="dself")
            pexp = acc.tile([128, 1], _f32, tag="pexp")
            eacc = acc.tile([128, NPAIR], _f32, tag="eacc")
            cacc = acc.tile([128, NPAIR], _f32, tag="cacc")
            # lhsT = own-block columns (permuted cols 512..1023) of this
            # m-tile; the same SBUF tiles feed both matmul operands.
            lo = RPC + 128 * t
            for g in range(2):  # two pair-tiles per group, 4 k-steps shared
                pA = psum.tile([128, PAIR], _f32, tag="ps", name="psA")
                pB = psum.tile([128, PAIR], _f32, tag="ps", name="psB")
                for k in range(KT):
                    lhsT = rhs_sb[k][:, lo : lo + 128]
                    for h, p in ((0, pA), (1, pA), (0, pB), (1, pB)):
                        c = 4 * g + (0 if p is pA else 2) + h
                        nc.tensor.matmul(
                            p[:, 512 * h : 512 * (h + 1)],
                            lhsT,
                            rhs_sb[k][:, 512 * c : 512 * (c + 1)],
                            start=(k == 0),
                            stop=(k == KT - 1),
                        )
                if g == 0:
                    # pair A of group 0 holds both diagonals: partner
                    # block at cols 0..511, own block at cols 512..1023.
                    # Identity-mask extraction is exact (one nonzero/row).
                    for off, dst in ((128 * t, pos), (512 + 128 * t, dself)):
                        dj = scratch.tile([128, 128], _f32, tag="diagjunk", bufs=4)
                        nc.vector.tensor_mul(dj[:], pA[:, off : off + 128], ident[:])
                        nc.vector.reduce_sum(
                            out=dst[:], in_=dj[:], axis=mybir.AxisListType.X
                        )
                    # threshold for the exp-domain compare: exp(k*pos)
                    nc.scalar.activation(
                        out=pexp[:],
                        in_=pos[:],
                        func=mybir.ActivationFunctionType.Exp,
                        scale=kscale[:],
                    )
                for idx, p in ((2 * g, pA), (2 * g + 1, pB)):
                    ej = scratch.tile([128, PAIR], _f32, tag="ej")
                    nc.scalar.activation(
                        out=ej[:],
                        in_=p[:],
                        func=mybir.ActivationFunctionType.Exp,
                        scale=kscale[:],
                        accum_out=eacc[:, idx : idx + 1],
                    )
                    # count in the exp domain (monotone): reads f32 SBUF
                    # at DVE 2x instead of PSUM at 1x. Partner element is
                    # bit-equal to pexp (same input, same LUT) -> not >.
                    cj = scratch.tile([128, PAIR], _bf16, tag="cj")
                    nc.vector.tensor_scalar(
                        out=cj[:],
                        in0=ej[:],
                        scalar1=pexp[:],
                        scalar2=None,
                        op0=mybir.AluOpType.is_gt,
                        op1=mybir.AluOpType.add,
                        accum_out=cacc[:, idx : idx + 1],
                    )
            nc.vector.reduce_sum(
                out=outs[:, 4 * t : 4 * t + 1], in_=eacc[:], axis=mybir.AxisListType.X
            )
            nc.vector.tensor_copy(out=outs[:, 4 * t + 1 : 4 * t + 2], in_=dself[:])
            nc.vector.tensor_copy(out=outs[:, 4 * t + 2 : 4 * t + 3], in_=pos[:])
            nc.vector.reduce_sum(
                out=outs[:, 4 * t + 3 : 4 * t + 4],
                in_=cacc[:],
                axis=mybir.AxisListType.X,
            )

        nc.sync.dma_start(out=out_d, in_=outs[:])


def _build_nc():
    if "nc" in _NC_CACHE:
        return _NC_CACHE["nc"]
    # Bacc (not raw Bass): its compile() runs generate_event_semaphores,
    # which splits multi-sem waits into EventSemaphore instructions — the
    # hardware allows at most one sync wait per compute instruction.
    nc = bacc.Bacc("TRN2")
    with tile.TileContext(nc) as tc:
        _emit(tc)
    nc.compile()
    _NC_CACHE["nc"] = nc
    return nc


LAST_RESULT = None


def kernel(z_i, z_j, temperature=0.5):
    global LAST_RESULT
    z_i = np.asarray(z_i, dtype=np.float32)
    z_j = np.asarray(z_j, dtype=np.float32)
    assert z_i.shape == (B, D) and z_j.shape == (B, D)

    z = np.concatenate([z_i, z_j], axis=0)
    nrm = np.sqrt((z.astype(np.float64) ** 2).sum(axis=1, keepdims=True))
    nrm = np.maximum(nrm, 1e-8)
    zn = z / nrm
    zq = (zn * S8).astype(ml_dtypes.float8_e4m3)
    znT = np.ascontiguousarray(zq.T)  # [D, N]

    # device computes exp(kscale * G); host converts G-unit outputs with k
    k = (1.0 / float(temperature)) / (S8 * S8)

    rows = np.arange(N)
    in_maps = []
    for r in range(NCORES):
        own = rows[r * RPC : (r + 1) * RPC]
        part = (own + B) % N
        rest_mask = np.ones(N, dtype=bool)
        rest_mask[own] = False
        rest_mask[part] = False
        perm = np.concatenate([part, own, rows[rest_mask]])
        rhs = np.ascontiguousarray(znT[:, perm]).reshape(KT, 128, N)
        in_maps.append({"rhs": rhs})

    nc = _build_nc()
    res = run_bass_kernel_spmd(nc, in_maps, core_ids=list(range(NCORES)))
    LAST_RESULT = res

    tot_loss = 0.0
    tot_rank = 0.0
    for r in range(NCORES):
        o = np.asarray(res.results[r]["out"], dtype=np.float64)  # [128, 17]
        for t in range(MT):
            S = o[:, 4 * t + 0]
            dG = o[:, 4 * t + 1]
            pG = o[:, 4 * t + 2]
            cnt = o[:, 4 * t + 3]
            d = dG * k
            p = pG * k
            Sc = S - np.exp(d)  # exclude the self term
            tot_loss += (np.log(Sc) - p).sum()
            tot_rank += (cnt - (dG > pG)).sum()

    loss = np.array(tot_loss / N, dtype=np.float32)
    avg_rank = np.array(tot_rank / N, dtype=np.float32)
    return loss, avg_rank


# revision 21
# speedup vs baseline: 1.0958x; 1.0958x over previous
"""NT-Xent (SimCLR) loss on 8 Trainium2 NeuronCores.

Math (validated against the reference formulation in f64):
  z = concat(z_i, z_j)                      [N=4096, D=512]
  zn = z / max(||z||, eps);  sim = zn@zn.T / T   (T=0.5, sim in [-2, 2])
  logits row i = sim row i minus the self-diagonal entry, so
    lse_i  = log(sum_{j!=i} exp(sim_ij))          (sim bounded => no shift)
    loss   = mean_i (lse_i - sim[i, partner(i)])
    rank_i = #{j != i : sim_ij > sim[i, partner(i)]}  (partner ties at 0)
    avg_rank = mean_i rank_i

Sharding: core r owns 512 rows of sim. Host normalizes z, quantizes
zn*S8 to fp8e4m3, transposes to [D, N] and permutes columns per core to
[partner-block | own-block | rest] so the partner / self diagonals sit
at fixed compile-time offsets (one NEFF for all cores; row stats are
column-permutation invariant). The gram G = (S8 zn)(S8 zn)^T carries
sim = k*G with k = (1/T)/S8^2 folded into the ScalarE activation scale.

PE runs fp8 DoubleRow matmuls (2 fp8 weights per cell -> 2x bf16
throughput): operands are laid out [p, i, n] with contraction index
K = s*256 + i*128 + p over two 256-deep super-tiles s, so each chunk
needs just 2 accumulating matmuls. Per 1024-col PSUM pair (2 banks):
one ScalarE exp with fused row-sum (accum_out), one VectorE
greater-than in the exp domain (f32 SBUF read) + bf16 row-sum reduce.
Diagonals (pos, self) are extracted exactly via identity-mask multiply
+ reduce from the first PSUM pair. Device emits per-row (S_full, d_G,
pos_G, count) in G units; the host applies the exact self-exclusion
corrections and the final log/mean in f64.
"""

import numpy as np
import ml_dtypes

import concourse.bacc as bacc
import concourse.mybir as mybir
import concourse.tile as tile
from concourse.bass_utils import run_bass_kernel_spmd
from concourse.masks import make_identity

B = 2048
D = 512
N = 2 * B
NCORES = 8
RPC = N // NCORES  # rows of sim per core = 512
ST = D // 256      # DoubleRow super-tiles = 2 (K = 256 each)
MT = RPC // 128    # m tiles per core = 4
PAIR = 1024        # columns per PSUM pair-tile (2 banks of f32)
NPAIR = N // PAIR  # pairs per m-tile = 4
S8 = 16.0          # fp8 pre-scale on zn
KSCALE = (1.0 / 0.5) / (S8 * S8)  # sim = KSCALE * G   (T = 0.5)

_f32 = mybir.dt.float32
_bf16 = mybir.dt.bfloat16
_fp8 = mybir.dt.float8e4

_NC_CACHE = {}


def _emit(tc):
    nc = tc.nc
    rhs_d = nc.dram_tensor("rhs", [ST, 128, 2, N], _fp8, kind="ExternalInput")[:]
    out_d = nc.dram_tensor("out", [128, 4 * MT + 1], _f32, kind="ExternalOutput")[:]

    with (
        tc.tile_pool(name="singles", bufs=1) as singles,
        tc.tile_pool(name="psum", bufs=4, space="PSUM") as psum,
        tc.tile_pool(name="scratch", bufs=3) as scratch,
        tc.tile_pool(name="acc", bufs=4) as acc,
    ):
        ident = singles.tile([128, 128], _f32)
        make_identity(nc, ident)
        kscale = singles.tile([128, 1], _f32)
        nc.vector.memset(kscale, KSCALE)

        # Stage the operand as 2 super-tiles of [128, 2, 4096] fp8
        # (8 KiB/partition each). q-major DMA order so the columns the
        # first matmuls need land first (both super-tiles' q=0).
        rhs_sb = [singles.tile([128, 2, N], _fp8, tag=f"rhs{s}", name=f"rhs{s}") for s in range(ST)]
        for q in range(4):
            for s in range(ST):
                nc.sync.dma_start(
                    out=rhs_sb[s][:, :, q * 1024 : (q + 1) * 1024],
                    in_=rhs_d[s, :, :, q * 1024 : (q + 1) * 1024],
                )

        # Spare column 16 is written once and ignored by the host.
        outs = singles.tile([128, 4 * MT + 1], _f32)
        nc.vector.tensor_scalar_mul(outs[:, 4 * MT : 4 * MT + 1], ident[:, 0:1], 0.0)

        for t in range(MT):
            pos = acc.tile([128, 1], _f32, tag="pos")
            dself = acc.tile([128, 1], _f32, tag="dself")
            pexp = acc.tile([128, 1], _f32, tag="pexp")
            eacc = acc.tile([128, NPAIR], _f32, tag="eacc")
            cacc = acc.tile([128, NPAIR], _f32, tag="cacc")
            # lhsT = own-block columns (permuted cols 512..1023) of this
            # m-tile; the same SBUF tiles feed both matmul operands.
            lo = RPC + 128 * t
            for g in range(2):
                pA = psum.tile([128, PAIR], _f32, tag="ps", name="psA")
                pB = psum.tile([128, PAIR], _f32, tag="ps", name="psB")
                for s in range(ST):
                    lhsT = rhs_sb[s][:, :, lo : lo + 128]
                    for h, p in ((0, pA), (1, pA), (0, pB), (1, pB)):
                        c = 4 * g + (0 if p is pA else 2) + h
                        nc.tensor.matmul(
                            p[:, 512 * h : 512 * (h + 1)],
                            lhsT,
                            rhs_sb[s][:, :, 512 * c : 512 * (c + 1)],
                            start=(s == 0),
                            stop=(s == ST - 1),
                            perf_mode=mybir.MatmulPerfMode.DoubleRow,
                        )
                if g == 0:
                    # pair A of group 0 holds both diagonals: partner
                    # block at cols 0..511, own block at cols 512..1023.
                    # Identity-mask extraction is exact (one nonzero/row).
                    for off, dst in ((128 * t, pos), (512 + 128 * t, dself)):
                        dj = scratch.tile([128, 128], _f32, tag="diagjunk", bufs=4)
                        nc.vector.tensor_mul(dj[:], pA[:, off : off + 128], ident[:])
                        nc.vector.reduce_sum(
                            out=dst[:], in_=dj[:], axis=mybir.AxisListType.X
                        )
                    # threshold for the exp-domain compare: exp(k*pos)
                    nc.scalar.activation(
                        out=pexp[:],
                        in_=pos[:],
                        func=mybir.ActivationFunctionType.Exp,
                        scale=kscale[:],
                    )
                for idx, p in ((2 * g, pA), (2 * g + 1, pB)):
                    ej = scratch.tile([128, PAIR], _f32, tag="ej")
                    nc.scalar.activation(
                        out=ej[:],
                        in_=p[:],
                        func=mybir.ActivationFunctionType.Exp,
                        scale=kscale[:],
                        accum_out=eacc[:, idx : idx + 1],
                    )
                    # count in the exp domain (monotone): reads f32 SBUF
                    # (DVE 2x port mode, no accum fusion) then reduces the
                    # bf16 0/1 tile at 4x. Partner element is bit-equal to
                    # pexp (same input, same LUT) -> not greater.
                    cj = scratch.tile([128, PAIR], _bf16, tag="cj")
                    nc.vector.tensor_scalar(
                        out=cj[:],
                        in0=ej[:],
                        scalar1=pexp[:],
                        scalar2=None,
                        op0=mybir.AluOpType.is_gt,
                    )
                    nc.vector.reduce_sum(
                        out=cacc[:, idx : idx + 1],
                        in_=cj[:],
                        axis=mybir.AxisListType.X,
                    )
            nc.vector.reduce_sum(
                out=outs[:, 4 * t : 4 * t + 1], in_=eacc[:], axis=mybir.AxisListType.X
            )
            nc.vector.tensor_copy(out=outs[:, 4 * t + 1 : 4 * t + 2], in_=dself[:])
            nc.vector.tensor_copy(out=outs[:, 4 * t + 2 : 4 * t + 3], in_=pos[:])
            nc.vector.reduce_sum(
                out=outs[:, 4 * t + 3 : 4 * t + 4],
                in_=cacc[:],
                axis=mybir.AxisListType.X,
            )

        nc.sync.dma_start(out=out_d, in_=outs[:])


def _build_nc():
    if "nc" in _NC_CACHE:
        return _NC_CACHE["nc"]
    # Bacc (not raw Bass): its compile() runs generate_event_semaphores,
    # which splits multi-sem waits into EventSemaphore instructions — the
    # hardware allows at most one sync wait per compute instruction.
    nc = bacc.Bacc("TRN2")
    with tile.TileContext(nc) as tc:
        _emit(tc)
    nc.compile()
    _NC_CACHE["nc"] = nc
    return nc


LAST_RESULT = None


def kernel(z_i, z_j, temperature=0.5):
    global LAST_RESULT
    z_i = np.asarray(z_i, dtype=np.float32)
    z_j = np.asarray(z_j, dtype=np.float32)
    assert z_i.shape == (B, D) and z_j.shape == (B, D)

    z = np.concatenate([z_i, z_j], axis=0)
    nrm = np.sqrt((z.astype(np.float64) ** 2).sum(axis=1, keepdims=True))
    nrm = np.maximum(nrm, 1e-8)
    zn = z / nrm
    zq = (zn * S8).astype(ml_dtypes.float8_e4m3)
    znT = np.ascontiguousarray(zq.T)  # [D, N]

    # device computes exp(kscale * G); host converts G-unit outputs with k
    k = (1.0 / float(temperature)) / (S8 * S8)

    rows = np.arange(N)
    in_maps = []
    for r in range(NCORES):
        own = rows[r * RPC : (r + 1) * RPC]
        part = (own + B) % N
        rest_mask = np.ones(N, dtype=bool)
        rest_mask[own] = False
        rest_mask[part] = False
        perm = np.concatenate([part, own, rows[rest_mask]])
        zp = znT[:, perm]  # [512, 4096]
        # DoubleRow layout: k = s*256 + i*128 + p  ->  [s, p, i, n]
        rhs = np.ascontiguousarray(
            zp.reshape(ST, 2, 128, N).transpose(0, 2, 1, 3)
        )
        in_maps.append({"rhs": rhs})

    nc = _build_nc()
    res = run_bass_kernel_spmd(nc, in_maps, core_ids=list(range(NCORES)))
    LAST_RESULT = res

    tot_loss = 0.0
    tot_rank = 0.0
    for r in range(NCORES):
        o = np.asarray(res.results[r]["out"], dtype=np.float64)  # [128, 17]
        for t in range(MT):
            S = o[:, 4 * t + 0]
            dG = o[:, 4 * t + 1]
            pG = o[:, 4 * t + 2]
            cnt = o[:, 4 * t + 3]
            d = dG * k
            p = pG * k
            Sc = S - np.exp(d)  # exclude the self term
            tot_loss += (np.log(Sc) - p).sum()
            tot_rank += (cnt - (dG > pG)).sum()

    loss = np.array(tot_loss / N, dtype=np.float32)
    avg_rank = np.array(tot_rank / N, dtype=np.float32)
    return loss, avg_rank
